# revision 1
# baseline (speedup 1.0000x reference)
"""CenterNet (CtdetLoss) Trainium2 Bass kernel.

Math: with p = pred_hm, t = log1p(-p) * p^2, m4 = (1-hm)^4,
  F - Z = t*(m4-1)  densely, plus  ln(p)*(1-p)^2  at the K-sparse
  positive pixels (hm == 1.0, which are exactly the object centers).
Per-object rectangle sums are computed without summed-area tables:
  rect_k(channel c_k) = sum_y My[k,y] * sum_x Mx[k,x] * G[c_k,y,x]
The y-contraction runs on the TensorEngine (lhsT = My^T 0/1 matrix,
4 channels per 512-wide matmul), the x-mask + reduce on the
VectorEngine (fused scalar_tensor_tensor accum), and the class selection is a
one-hot mask reduce over the accumulated [K, C] table. The class-summed
Z map for S_ZS is accumulated over channel groups and pushed through the
same masked-matmul once per image. Positive-pixel values and the reg-L1
values are fetched with indirect row-gather DMAs (512B rows) and
column-selected with shipped one-hot masks.

Sharding: data-parallel over batch, 2 images per core on 8 cores. Host
preprocessing only touches the small int tensors (masks, one-hots,
gather row indices, per-object weights); every FLOP on dense map data
runs on device. Host combines the 8 cores' per-image partial sums into
the final 4 scalars.
"""

import os
import sys

sys.path.insert(0, "/opt/trn_rl_repo")

import numpy as np
import ml_dtypes

B, C, H, W, K = 16, 80, 128, 128, 128
NCORES = 8
NB = B // NCORES          # images per core
CG = 8                    # channels per group
NG = C // CG              # channel groups
HM_W, WH_W, OFF_W = 1.0, 0.1, 1.0

BF16 = ml_dtypes.bfloat16

_module_cache = {}


def build_module():
    """Build (once) the per-core Bass module. Returns (nc, out_name)."""
    if "nc" in _module_cache:
        return _module_cache["nc"]

    import concourse.bacc as bacc
    import concourse.bass as bass
    import concourse.tile as tile
    from concourse import mybir

    f32 = mybir.dt.float32
    bf16 = mybir.dt.bfloat16
    i32 = mybir.dt.int32
    Alu = mybir.AluOpType
    Act = mybir.ActivationFunctionType
    Ax = mybir.AxisListType

    nc = bacc.Bacc(None, target_bir_lowering=False)

    # ---- DRAM I/O ----
    phm = nc.dram_tensor("phm", [NB, C, H, W], f32, kind="ExternalInput")
    hmt = nc.dram_tensor("hm", [NB, C, H, W], f32, kind="ExternalInput")
    pwh = nc.dram_tensor("pwh", [NB, 2, H, W], f32, kind="ExternalInput")
    prg = nc.dram_tensor("prg", [NB, 2, H, W], f32, kind="ExternalInput")
    mytb = nc.dram_tensor("mytb", [NB, H, K], bf16, kind="ExternalInput")
    mxr = nc.dram_tensor("mxr", [NB, K, 4 * W], f32, kind="ExternalInput")
    eoh = nc.dram_tensor("eoh", [NB, K, C], f32, kind="ExternalInput")
    sk = nc.dram_tensor("sk", [NB, K, 1], f32, kind="ExternalInput")
    mts = nc.dram_tensor("mts", [NB, K, K], f32, kind="ExternalInput")
    rpos = nc.dram_tensor("rpos", [NB, K, 1], i32, kind="ExternalInput")
    cxsel = nc.dram_tensor("cxsel", [NB, K, W], f32, kind="ExternalInput")
    rwh = nc.dram_tensor("rwh", [NB, 2, K, 1], i32, kind="ExternalInput")
    rrg = nc.dram_tensor("rrg", [NB, 2, K, 1], i32, kind="ExternalInput")
    csind = nc.dram_tensor("csind", [NB, K, W], f32, kind="ExternalInput")
    m2 = nc.dram_tensor("m2", [NB, K, 2], f32, kind="ExternalInput")
    tmw = nc.dram_tensor("tmw", [NB, K, 2], f32, kind="ExternalInput")
    tmr = nc.dram_tensor("tmr", [NB, K, 2], f32, kind="ExternalInput")
    out = nc.dram_tensor("out", [4, NB], f32, kind="ExternalOutput")

    phm_flat = phm[:].rearrange("b c y x -> (b c y) x")
    pwh_flat = pwh[:].rearrange("b d y x -> (b d y) x")
    prg_flat = prg[:].rearrange("b d y x -> (b d y) x")

    with tile.TileContext(nc) as tc:
        with (
            tc.tile_pool(name="consts", bufs=1) as consts,
            tc.tile_pool(name="work", bufs=3) as work,
            tc.tile_pool(name="scr", bufs=4) as scr,
            tc.tile_pool(name="acc", bufs=1) as acc,
            tc.tile_pool(name="ep", bufs=2) as ep,
            tc.tile_pool(name="psb", bufs=2, space="PSUM") as psb,
            tc.tile_pool(name="psz", bufs=1, space="PSUM") as pszp,
            tc.tile_pool(name="pss", bufs=1, space="PSUM") as pss,
        ):
            ones_s = consts.tile([K, 1], f32, tag="ones")
            nc.vector.memset(ones_s, 1.0)
            O = acc.tile([4, NB], f32, tag="O")

            for b in range(NB):
                # ---- per-image constants ----
                myt_s = consts.tile([H, K], bf16, tag=f"myt{b}")
                nc.sync.dma_start(out=myt_s, in_=mytb[b])
                mxr_s = consts.tile([K, 4 * W], f32, tag=f"mxr{b}")
                nc.sync.dma_start(out=mxr_s, in_=mxr[b])
                eoh_s = consts.tile([K, C], f32, tag=f"eoh{b}")
                nc.sync.dma_start(out=eoh_s, in_=eoh[b])
                sk_s = consts.tile([K, 1], f32, tag=f"sk{b}")
                nc.sync.dma_start(out=sk_s, in_=sk[b])
                mt_s = consts.tile([K, K], f32, tag=f"mt{b}")
                nc.sync.dma_start(out=mt_s, in_=mts[b])
                rpos_s = consts.tile([K, 1], i32, tag=f"rpos{b}")
                nc.sync.dma_start(out=rpos_s, in_=rpos[b])
                cxsel_s = consts.tile([K, W], f32, tag=f"cxsel{b}")
                nc.sync.dma_start(out=cxsel_s, in_=cxsel[b])
                rwh_s = [
                    consts.tile([K, 1], i32, tag=f"rwh{b}{d}", name=f"rwh_s{b}{d}")
                    for d in range(2)
                ]
                rrg_s = [
                    consts.tile([K, 1], i32, tag=f"rrg{b}{d}", name=f"rrg_s{b}{d}")
                    for d in range(2)
                ]
                for d in range(2):
                    nc.sync.dma_start(out=rwh_s[d], in_=rwh[b, d])
                    nc.sync.dma_start(out=rrg_s[d], in_=rrg[b, d])
                csind_s = consts.tile([K, W], f32, tag=f"csind{b}")
                nc.sync.dma_start(out=csind_s, in_=csind[b])
                m2_s = consts.tile([K, 2], f32, tag=f"m2{b}")
                nc.sync.dma_start(out=m2_s, in_=m2[b])
                tmw_s = consts.tile([K, 2], f32, tag=f"tmw{b}")
                nc.sync.dma_start(out=tmw_s, in_=tmw[b])
                tmr_s = consts.tile([K, 2], f32, tag=f"tmr{b}")
                nc.sync.dma_start(out=tmr_s, in_=tmr[b])

                V = acc.tile([K, C], f32, tag=f"V{b}")
                psz_acc = pszp.tile([K, 4 * W], f32, tag=f"pszacc{b}")

                # ---- dense channel-group loop ----
                for g in range(NG):
                    cs = g * CG
                    p4 = work.tile([H, CG * W], f32, tag="p4")
                    nc.sync.dma_start(
                        out=p4[:].rearrange("p (c x) -> p c x", c=CG),
                        in_=phm[b, cs : cs + CG].rearrange("c y x -> y c x"),
                    )
                    hm4 = work.tile([H, CG * W], f32, tag="hm4")
                    nc.sync.dma_start(
                        out=hm4[:].rearrange("p (c x) -> p c x", c=CG),
                        in_=hmt[b, cs : cs + CG].rearrange("c y x -> y c x"),
                    )
                    # t = log1p(-p) * p^2 ; m4 = (1-hm)^4 ; g4 = (m4-1)*t
                    # ACT: l1, p2, m2t, m4 | Pool: t | DVE: g4 + mask reduces
                    l1 = work.tile([H, CG * W], f32, tag="l1")
                    nc.scalar.activation(l1, p4, Act.Ln, bias=1.0, scale=-1.0)
                    p2 = work.tile([H, CG * W], f32, tag="p2")
                    nc.scalar.activation(p2, p4, Act.Square)
                    t = work.tile([H, CG * W], bf16, tag="t")
                    nc.gpsimd.tensor_mul(t, l1, p2)
                    m2t = work.tile([H, CG * W], f32, tag="m2t")
                    nc.scalar.activation(m2t, hm4, Act.Square, bias=1.0, scale=-1.0)
                    m4t = work.tile([H, CG * W], f32, tag="m4t")
                    nc.scalar.activation(m4t, m2t, Act.Square)
                    g4 = work.tile([H, CG * W], bf16, tag="g4")
                    nc.vector.scalar_tensor_tensor(
                        g4, m4t, -1.0, t, op0=Alu.add, op1=Alu.mult
                    )
                    # S_ZS accumulation on PE: psz_acc += MyT.T @ t (both halves
                    # fold onto the same 512 columns; residues sum out in the
                    # final Mx mask reduce)
                    for hh in range(2):
                        nc.tensor.matmul(
                            psz_acc, lhsT=myt_s, rhs=t[:, hh * 512 : hh * 512 + 512],
                            start=(g == 0 and hh == 0),
                            stop=(g == NG - 1 and hh == 1),
                            skip_group_check=True,
                        )

                    psg = psb.tile([K, CG * W], f32, tag="psg")
                    for hh in range(2):
                        nc.tensor.matmul(
                            psg[:, hh * 512 : hh * 512 + 512], lhsT=myt_s,
                            rhs=g4[:, hh * 512 : hh * 512 + 512],
                            start=True, stop=True, skip_group_check=True,
                        )
                    for cc in range(CG):
                        sl = slice(cc * W, (cc + 1) * W)
                        sc = scr.tile([K, W], f32, tag="scr128")
                        nc.vector.scalar_tensor_tensor(
                            sc, psg[:, sl], 1.0, mxr_s[:, (cc % 4) * W : (cc % 4) * W + W],
                            op0=Alu.mult, op1=Alu.mult,
                            accum_out=V[:, cs + cc : cs + cc + 1],
                        )

                # ---- per-image epilogue ----
                # S_ZS: Mx-masked reduce of the PE-accumulated psz_acc
                szs = ep.tile([K, 1], f32, tag="szs")
                sc512 = scr.tile([K, 4 * W], f32, tag="scr512")
                nc.vector.scalar_tensor_tensor(
                    sc512, psz_acc, 1.0, mxr_s,
                    op0=Alu.mult, op1=Alu.mult, accum_out=szs,
                )
                # class-select rect sums: rectG[k] = sum_c V[k,c] * onehot[k,c]
                rectG = ep.tile([K, 1], f32, tag="rectG")
                sc80 = scr.tile([K, C], f32, tag="scr80")
                nc.vector.scalar_tensor_tensor(
                    sc80, V, 1.0, eoh_s,
                    op0=Alu.mult, op1=Alu.mult, accum_out=rectG,
                )
                # positive-pixel term: gather pred_hm rows at the unique
                # centers, column-select, A = ln(p)*(1-p)^2, posG = MT.T @ A
                rowg = ep.tile([K, W], f32, tag="rowg")
                nc.gpsimd.indirect_dma_start(
                    out=rowg,
                    out_offset=None,
                    in_=phm_flat,
                    in_offset=bass.IndirectOffsetOnAxis(ap=rpos_s[:], axis=0),
                )
                pj = ep.tile([K, 1], f32, tag="pj")
                sc = scr.tile([K, W], f32, tag="scr128")
                nc.vector.scalar_tensor_tensor(
                    sc, rowg, 1.0, cxsel_s,
                    op0=Alu.mult, op1=Alu.mult, accum_out=pj,
                )
                lnp = ep.tile([K, 1], f32, tag="lnp")
                nc.scalar.activation(lnp, pj, Act.Ln)
                q2 = ep.tile([K, 1], f32, tag="q2")
                nc.scalar.activation(q2, pj, Act.Square, bias=1.0, scale=-1.0)
                A = ep.tile([K, 1], f32, tag="A")
                nc.vector.tensor_mul(A, lnp, q2)
                psp = pss.tile([K, 1], f32, tag="psp")
                nc.tensor.matmul(psp, lhsT=mt_s, rhs=A, start=True, stop=True)
                # total = rectG + posG + S_ZS ;  Q[:,0] = total * s
                tot = ep.tile([K, 1], f32, tag="tot")
                nc.vector.tensor_add(tot, rectG, psp)
                nc.vector.tensor_add(tot, tot, szs)
                Q = ep.tile([K, 4], f32, tag=f"Q{b}")
                nc.vector.memset(Q, 0.0)
                nc.vector.tensor_mul(Q[:, 0:1], tot, sk_s)
                # reg-L1 columns
                for col, flat, rows, tm in (
                    (1, pwh_flat, rwh_s, tmw_s),
                    (2, prg_flat, rrg_s, tmr_s),
                ):
                    PW = ep.tile([K, 2], f32, tag=f"PW{col}")
                    for d in range(2):
                        rg = ep.tile([K, W], f32, tag=f"rg{col}{d}")
                        nc.gpsimd.indirect_dma_start(
                            out=rg,
                            out_offset=None,
                            in_=flat,
                            in_offset=bass.IndirectOffsetOnAxis(
                                ap=rows[d][:], axis=0
                            ),
                        )
                        sc = scr.tile([K, W], f32, tag="scr128")
                        nc.vector.scalar_tensor_tensor(
                            sc, rg, 1.0, csind_s,
                            op0=Alu.mult, op1=Alu.mult,
                            accum_out=PW[:, d : d + 1],
                        )
                    u = ep.tile([K, 2], f32, tag=f"u{col}")
                    nc.vector.tensor_mul(u, PW, m2_s)
                    nc.vector.tensor_sub(u, u, tm)
                    nc.vector.tensor_reduce(
                        Q[:, col : col + 1], u, axis=Ax.X, op=Alu.add,
                        apply_absolute_value=True,
                    )
                # partition-reduce the 4 columns: out[4,1] = Q.T @ ones
                psq = pss.tile([4, 1], f32, tag="psq")
                nc.tensor.matmul(psq, lhsT=Q, rhs=ones_s, start=True, stop=True)
                nc.scalar.copy(O[:, b : b + 1], psq)

            nc.sync.dma_start(out=out[:], in_=O)

    nc.compile()
    _module_cache["nc"] = nc
    return nc


def prep_in_maps(inputs):
    """Host-side prep: shard the dense maps per core, derive mask/index
    constants from the small int tensors."""
    pred_hm = np.asarray(inputs["pred_hm"], np.float32)
    pred_wh = np.asarray(inputs["pred_wh"], np.float32)
    pred_reg = np.asarray(inputs["pred_reg"], np.float32)
    hm = np.asarray(inputs["hm"], np.float32)
    wh_t = np.asarray(inputs["wh_t"], np.float32)
    reg_t = np.asarray(inputs["reg_t"], np.float32)
    reg_mask = np.asarray(inputs["reg_mask"], np.float32)
    ind = np.asarray(inputs["ind"]).astype(np.int64)
    cxcy = np.asarray(inputs["cxcy"]).astype(np.int64)
    ori_wh = np.asarray(inputs["ori_wh"]).astype(np.int64)
    cls_idx = np.asarray(inputs["cls_idx"]).astype(np.int64)

    yy = np.arange(H)
    xx = np.arange(W)
    per_img = []
    for b in range(B):
        cls = cls_idx[b]
        cx, cy = cxcy[b, :, 0], cxcy[b, :, 1]
        w = wh_t[b, :, 0].astype(np.int64)
        h = wh_t[b, :, 1].astype(np.int64)
        y0 = np.maximum(1, cy - h // 2 - 1)
        y1 = np.minimum(H - 1, cy + h // 2 + 1)
        y1 = np.maximum(y1, y0)
        x0 = np.maximum(1, cx - w // 2 - 1)
        x1 = np.minimum(W - 1, cx + w // 2 + 1)
        x1 = np.maximum(x1, x0)

        MyT = ((yy[:, None] >= y0[None, :]) & (yy[:, None] < y1[None, :]))
        Mx = ((xx[None, :] >= x0[:, None]) & (xx[None, :] < x1[:, None]))
        MxR = np.tile(Mx.astype(np.float32), (1, 4))
        Eoh = (cls[:, None] == np.arange(C)[None, :]).astype(np.float32)

        aspect = w.astype(np.float32) / h.astype(np.float32)
        ori = ori_wh[b, :, 0].astype(np.float32) / ori_wh[b, :, 1].astype(np.float32)
        bad = ~((aspect > 0.5 * ori) & (aspect < 2.0 * ori))
        badw = np.where(bad, 0.5, 1.0).astype(np.float32)
        valid = reg_mask[b] * (w * h > 0).astype(np.float32)

        # unique positive pixels (duplicated centers collapse in hm)
        flat = cls * (H * W) + cy * W + cx
        _, uidx = np.unique(flat, return_index=True)
        nu = len(uidx)
        cls_u, cy_u, cx_u = cls[uidx], cy[uidx], cx[uidx]
        inY = (cy_u[None, :] >= y0[:, None]) & (cy_u[None, :] < y1[:, None])
        inX = (cx_u[None, :] >= x0[:, None]) & (cx_u[None, :] < x1[:, None])
        sameC = cls[:, None] == cls_u[None, :]
        Mkj = (sameC & inY & inX).astype(np.float32)  # [k, j<nu]
        npos = Mkj.sum(1)
        MT = np.zeros((K, K), np.float32)
        MT[:nu, :] = Mkj.T
        rpos_v = np.zeros((K, 1), np.int32)
        rpos_v[:nu, 0] = (b % NB) * C * H + cls_u * H + cy_u
        cxsel_v = np.zeros((K, W), np.float32)
        cx_pad = np.zeros(K, np.int64)
        cx_pad[:nu] = cx_u
        cxsel_v[np.arange(K), cx_pad] = 1.0

        r = np.where(npos > 0, 1.0 / np.maximum(npos, 1.0), 1.0)
        s = (-(r * badw * valid)).astype(np.float32)

        rind = ind[b] // W
        cind = ind[b] % W
        rwh_v = np.zeros((2, K, 1), np.int32)
        rrg_v = np.zeros((2, K, 1), np.int32)
        for d in range(2):
            rwh_v[d, :, 0] = (b % NB) * 2 * H + d * H + rind
            rrg_v[d, :, 0] = (b % NB) * 2 * H + d * H + rind
        csind_v = np.zeros((K, W), np.float32)
        csind_v[np.arange(K), cind] = 1.0

        m = reg_mask[b]
        M2 = np.stack([m, m], 1).astype(np.float32)
        TMW = (wh_t[b] * m[:, None]).astype(np.float32)
        TMR = (reg_t[b] * m[:, None]).astype(np.float32)
        nobj = float(m.sum())
        c1 = (1.0 / max(nobj, 1.0)) if nobj > 0 else 1.0
        invden = 1.0 / (2.0 * nobj + 1e-4)

        per_img.append(
            dict(
                MyT=MyT.astype(BF16), MxR=MxR, Eoh=Eoh, s=s.reshape(K, 1),
                MT=MT, rpos=rpos_v, cxsel=cxsel_v, rwh=rwh_v, rrg=rrg_v,
                csind=csind_v, M2=M2, TMW=TMW, TMR=TMR, c1=c1, invden=invden,
            )
        )

    in_maps = []
    for core in range(NCORES):
        bs = [core * NB + j for j in range(NB)]
        pi = [per_img[b] for b in bs]
        in_maps.append(
            {
                "phm": np.ascontiguousarray(pred_hm[bs]),
                "hm": np.ascontiguousarray(hm[bs]),
                "pwh": np.ascontiguousarray(pred_wh[bs]),
                "prg": np.ascontiguousarray(pred_reg[bs]),
                "mytb": np.stack([p["MyT"] for p in pi]),
                "mxr": np.stack([p["MxR"] for p in pi]),
                "eoh": np.stack([p["Eoh"] for p in pi]),
                "sk": np.stack([p["s"] for p in pi]),
                "mts": np.stack([p["MT"] for p in pi]),
                "rpos": np.stack([p["rpos"] for p in pi]),
                "cxsel": np.stack([p["cxsel"] for p in pi]),
                "rwh": np.stack([p["rwh"] for p in pi]),
                "rrg": np.stack([p["rrg"] for p in pi]),
                "csind": np.stack([p["csind"] for p in pi]),
                "m2": np.stack([p["M2"] for p in pi]),
                "tmw": np.stack([p["TMW"] for p in pi]),
                "tmr": np.stack([p["TMR"] for p in pi]),
            }
        )
    aux = dict(
        c1=np.array([p["c1"] for p in per_img]),
        invden=np.array([p["invden"] for p in per_img]),
    )
    return in_maps, aux


def combine_outputs(outs, aux):
    """outs: list of 8 per-core 'out' arrays [4, NB]."""
    q = np.concatenate([o.T for o in outs], 0).astype(np.float64)  # [B, 4]
    q_hm, q_wh, q_rg = q[:, 0], q[:, 1], q[:, 2]
    wh_i = q_wh * aux["invden"]
    off_i = q_rg * aux["invden"]
    final_loss = np.mean(HM_W * q_hm + WH_W * wh_i + OFF_W * off_i)
    final_hm = np.mean(q_hm * aux["c1"])
    final_wh = np.mean(wh_i)
    final_off = np.mean(off_i)
    return (
        np.float32(final_loss),
        np.float32(final_hm),
        np.float32(final_wh),
        np.float32(final_off),
    )


def kernel(**inputs):
    from concourse.bass_utils import run_bass_kernel_spmd

    nc = build_module()
    in_maps, aux = prep_in_maps(inputs)
    res = run_bass_kernel_spmd(nc, in_maps, core_ids=list(range(NCORES)))
    outs = [r["out"] for r in res.results]
    return combine_outputs(outs, aux)



# revision 6
# speedup vs baseline: 1.2027x; 1.2027x over previous
"""CenterNet (CtdetLoss) Trainium2 Bass kernel.

Math: with p = pred_hm, t = log1p(-p) * p^2, m4 = (1-hm)^4,
  F - Z = t*(m4-1)  densely, plus  ln(p)*(1-p)^2  at the K-sparse
  positive pixels (hm == 1.0, which are exactly the object centers).
Per-object rectangle sums without summed-area tables:
  rect_k(channel c_k) = sum_y My[k,y] * sum_x Mx[k,x] * G[c_k,y,x]
The y-contraction runs on the TensorEngine (lhsT = My^T, 4 matmuls of
512 columns per 16-channel dense tile).  The per-class [K, C*W] table
of y-contracted sums is staged in SBUF (bf16); the per-object class
selection is ONE SWDGE dma_gather (idx = cls*128 + k) which lands the
selected rows TRANSPOSED as [x, k]; the x-mask reduce is then a single
128x128 multiply + a ones-matmul on the PE.  The class-summed Z map
for S_ZS accumulates on the PE across the whole image (x-folded onto
512 columns) and is mask-reduced once per image.

Engine split per dense tile [128 x 2048] (16 channels):
  ScalarE: l1=Ln(1-p), p2=Square(p), m2=Square(1-hm)   (3 acts)
  GpSimd:  t = l1*p2                                    (bf16 mul)
  VectorE: m4 = m2*m2, g4 = (m4-1)*t, psum->table copy  (bf16 2x)
  TensorE: 4 psz matmuls + 4 psg matmuls (bf16, N=512)

DMA: host pre-transposes pred_hm/hm into [NB, 5, H, 16, W] so each
dense tile is ONE fully-contiguous 1MB (f32) / 0.5MB (bf16) transfer
(8KB/4KB per-partition runs).  hm ships as bf16 (weight-only use via
(1-hm)^4; verified <2e-3 effect on rect sums).  Positive-pixel values
and the reg-L1 rows are fetched with indirect row-gather DMAs; the
four reg-L1 gathers are merged into one 2KB-row gather from a host-
packed [NB, H, 4, W] tensor.

Sharding: data-parallel over batch, 2 images per core on 8 cores.
Host combines the 8 cores' per-image partial sums into the 4 scalars.
"""

import sys

sys.path.insert(0, "/opt/trn_rl_repo")

import numpy as np
import ml_dtypes

B, C, H, W, K = 16, 80, 128, 128, 128
NCORES = 8
NB = B // NCORES          # images per core
DG = 16                   # channels per dense tile
NDG = C // DG             # dense tiles per image
HM_W, WH_W, OFF_W = 1.0, 0.1, 1.0

BF16 = ml_dtypes.bfloat16

# const-pack column layout (f32 columns, per image, partition dim 128)
_CST_COLS = dict(
    myt=(0, 64),      # bf16 [H,128] lhsT My^T
    mxr=(64, 256),    # bf16 [K,512] Mx tiled x4 (S_ZS mask)
    mxt=(320, 64),    # bf16 [W,128] Mx^T (rect-sel mask, x on partitions)
    mts=(384, 64),    # bf16 [K,128] MT (pos-pixel distribution matrix)
    cxsel=(448, 64),  # bf16 [K,128] one-hot cx of unique positives
    csind=(512, 64),  # bf16 [K,128] one-hot x of reg-L1 centers
    sk=(576, 1),      # f32 [K,1] -(badw*valid*r)
    m2m=(577, 2),     # f32 [K,2] reg_mask pair
    tmw=(579, 2),     # f32 [K,2] wh_t*mask
    tmr=(581, 2),     # f32 [K,2] reg_t*mask
    rpos=(583, 1),    # i32 [K,1] pos-pixel row
    rwr=(584, 1),     # i32 [K,1] reg-L1 packed row
    gidx=(585, 4),    # i16 [128,8] dma_gather indices
)
CST_N = 592

_module_cache = {}


def build_module():
    if "nc" in _module_cache:
        return _module_cache["nc"]

    import concourse.bacc as bacc
    import concourse.bass as bass
    import concourse.tile as tile
    from concourse import mybir

    f32 = mybir.dt.float32
    bf16 = mybir.dt.bfloat16
    i32 = mybir.dt.int32
    i16 = mybir.dt.int16
    Alu = mybir.AluOpType
    Act = mybir.ActivationFunctionType
    Ax = mybir.AxisListType

    nc = bacc.Bacc(None, target_bir_lowering=False)

    # ---- DRAM I/O ----
    phm = nc.dram_tensor("phm", [NB, NDG, H, DG, W], f32, kind="ExternalInput")
    hmt = nc.dram_tensor("hm", [NB, NDG, H, DG, W], bf16, kind="ExternalInput")
    pwr = nc.dram_tensor("pwr", [NB, H, 4, W], f32, kind="ExternalInput")
    cst = nc.dram_tensor("cst", [NB, 128, CST_N], f32, kind="ExternalInput")
    out = nc.dram_tensor("out", [4, NB], f32, kind="ExternalOutput")

    phm_flat = phm[:].rearrange("b g y c x -> (b g y c) x")
    pwr_flat = pwr[:].rearrange("b y d x -> (b y) (d x)")

    def cs_f32(tile_, name):
        o, n = _CST_COLS[name]
        return tile_[:, o : o + n]

    def cs_bf16(tile_, name):
        o, n = _CST_COLS[name]
        return tile_[:, o : o + n].bitcast(bf16)

    def cs_i32(tile_, name):
        o, n = _CST_COLS[name]
        return tile_[:, o : o + n].bitcast(i32)

    with tile.TileContext(nc) as tc:
        with (
            tc.tile_pool(name="consts", bufs=1) as consts,
            tc.tile_pool(name="vtab", bufs=1) as vtab,
            tc.tile_pool(name="io", bufs=2) as iop,
            tc.tile_pool(name="work", bufs=2) as work,
            tc.tile_pool(name="scr", bufs=2) as scr,
            tc.tile_pool(name="acc", bufs=1) as acc,
            tc.tile_pool(name="ep", bufs=2) as ep,
            tc.tile_pool(name="psg", bufs=1, space="PSUM") as psgp,
            tc.tile_pool(name="psz", bufs=1, space="PSUM") as pszp,
            tc.tile_pool(name="pse", bufs=1, space="PSUM") as psep,
        ):
            ones_f = consts.tile([K, 1], f32, tag="onesf")
            nc.vector.memset(ones_f, 1.0)
            ones_b = consts.tile([K, 1], bf16, tag="onesb")
            nc.vector.memset(ones_b, 1.0)
            O = acc.tile([4, NB], f32, tag="O")

            cst_s = [
                consts.tile([128, CST_N], f32, tag=f"cst{b}", name=f"cst_s{b}")
                for b in range(NB)
            ]
            for b in range(NB):
                nc.sync.dma_start(out=cst_s[b], in_=cst[b])
            vt = [
                vtab.tile([K, C * W], bf16, tag=f"vt{b}", name=f"vt{b}")
                for b in range(NB)
            ]

            for b in range(NB):
                myt = cs_bf16(cst_s[b], "myt")
                psz_acc = pszp.tile([K, 4 * W], f32, tag="psz")

                # ---- dense tile loop: 16 channels per iteration ----
                for dg in range(NDG):
                    p4 = iop.tile([H, DG * W], f32, tag="p4")
                    nc.sync.dma_start(
                        out=p4, in_=phm[b, dg].rearrange("y c x -> y (c x)")
                    )
                    hm4 = iop.tile([H, DG * W], bf16, tag="hm4")
                    nc.sync.dma_start(
                        out=hm4, in_=hmt[b, dg].rearrange("y c x -> y (c x)")
                    )
                    l1 = work.tile([H, DG * W], bf16, tag="l1")
                    nc.scalar.activation(l1, p4, Act.Ln, bias=1.0, scale=-1.0)
                    p2 = work.tile([H, DG * W], bf16, tag="p2")
                    nc.scalar.activation(p2, p4, Act.Square)
                    m2 = work.tile([H, DG * W], bf16, tag="m2")
                    nc.scalar.activation(m2, hm4, Act.Square, bias=1.0, scale=-1.0)
                    t = work.tile([H, DG * W], bf16, tag="t")
                    nc.gpsimd.tensor_mul(t, l1, p2)
                    m4 = work.tile([H, DG * W], bf16, tag="m4")
                    nc.vector.tensor_mul(m4, m2, m2)
                    g4 = work.tile([H, DG * W], bf16, tag="g4")
                    nc.vector.scalar_tensor_tensor(
                        g4, m4, -1.0, t, op0=Alu.add, op1=Alu.mult
                    )
                    # S_ZS accumulation: psz_acc += MyT.T @ t, x-folded
                    for h in range(4):
                        nc.tensor.matmul(
                            psz_acc,
                            lhsT=myt,
                            rhs=t[:, h * 512 : h * 512 + 512],
                            start=(dg == 0 and h == 0),
                            stop=(dg == NDG - 1 and h == 3),
                            skip_group_check=True,
                        )
                    # per-class rect y-contraction
                    psg = psgp.tile([K, DG * W], f32, tag="psg")
                    for h in range(4):
                        nc.tensor.matmul(
                            psg[:, h * 512 : h * 512 + 512],
                            lhsT=myt,
                            rhs=g4[:, h * 512 : h * 512 + 512],
                            start=True,
                            stop=True,
                            skip_group_check=True,
                        )
                    nc.vector.tensor_copy(
                        out=vt[b][:, dg * DG * W : (dg + 1) * DG * W], in_=psg
                    )

                # ---- per-image epilogue ----
                # class-select: PT[x,k] = vt[k, cls[k]*W + x]
                PT = ep.tile([128, 1, K], bf16, tag="PT")
                nc.gpsimd.dma_gather(
                    out_ap=PT,
                    in_ap=vt[b][:],
                    idxs_ap=cs_f32(cst_s[b], "gidx").bitcast(i16),
                    num_idxs=K,
                    num_idxs_reg=K,
                    elem_size=W,
                    transpose=True,
                    sbuf_tokens_per_rank=128,
                    sbuf_free_dim_per_rank=W * 2,
                    sbuf_free_dim_pad_per_rank=0,
                    sbuf_byte_offset=0,
                )
                E = ep.tile([128, K], bf16, tag="E")
                nc.vector.tensor_mul(E, PT[:, 0], cs_bf16(cst_s[b], "mxt"))
                psel = psep.tile([K, 1], f32, tag="psel")
                nc.tensor.matmul(psel, lhsT=E, rhs=ones_b, start=True, stop=True)
                # S_ZS: Mx-masked reduce of the accumulated psz
                szs = ep.tile([K, 1], f32, tag="szs")
                sc512 = scr.tile([K, 4 * W], f32, tag="sc512")
                nc.vector.scalar_tensor_tensor(
                    sc512, psz_acc, 1.0, cs_bf16(cst_s[b], "mxr"),
                    op0=Alu.mult, op1=Alu.mult, accum_out=szs,
                )
                # positive pixels: gather rows, column-select, A=ln(p)(1-p)^2
                rowg = ep.tile([K, W], f32, tag="rowg")
                nc.gpsimd.indirect_dma_start(
                    out=rowg,
                    out_offset=None,
                    in_=phm_flat,
                    in_offset=bass.IndirectOffsetOnAxis(
                        ap=cs_i32(cst_s[b], "rpos"), axis=0
                    ),
                )
                pj = ep.tile([K, 1], f32, tag="pj")
                scw = scr.tile([K, W], f32, tag="scw")
                nc.vector.scalar_tensor_tensor(
                    scw, rowg, 1.0, cs_bf16(cst_s[b], "cxsel"),
                    op0=Alu.mult, op1=Alu.mult, accum_out=pj,
                )
                lnp = ep.tile([K, 1], f32, tag="lnp")
                nc.scalar.activation(lnp, pj, Act.Ln)
                q2 = ep.tile([K, 1], f32, tag="q2")
                nc.scalar.activation(q2, pj, Act.Square, bias=1.0, scale=-1.0)
                A = ep.tile([K, 1], bf16, tag="A")
                nc.vector.tensor_mul(A, lnp, q2)
                psp = psep.tile([K, 1], f32, tag="psp")
                nc.tensor.matmul(
                    psp, lhsT=cs_bf16(cst_s[b], "mts"), rhs=A, start=True, stop=True
                )
                # total * s -> Q[:,0]
                tot = ep.tile([K, 1], f32, tag="tot")
                nc.vector.tensor_add(tot, szs, psel)
                nc.vector.tensor_add(tot, tot, psp)
                Q = ep.tile([K, 4], f32, tag="Q")
                nc.vector.memset(Q, 0.0)
                nc.vector.tensor_mul(Q[:, 0:1], tot, cs_f32(cst_s[b], "sk"))
                # reg-L1: one 2KB-row gather covers wh d0,d1 + reg d0,d1
                rg = ep.tile([K, 4 * W], f32, tag="rg")
                nc.gpsimd.indirect_dma_start(
                    out=rg,
                    out_offset=None,
                    in_=pwr_flat,
                    in_offset=bass.IndirectOffsetOnAxis(
                        ap=cs_i32(cst_s[b], "rwr"), axis=0
                    ),
                )
                for col, base in ((1, 0), (2, 2)):
                    PW = ep.tile([K, 2], f32, tag=f"PW{col}")
                    for d in range(2):
                        sl = slice((base + d) * W, (base + d + 1) * W)
                        scw2 = scr.tile([K, W], f32, tag="scw")
                        nc.vector.scalar_tensor_tensor(
                            scw2, rg[:, sl], 1.0, cs_bf16(cst_s[b], "csind"),
                            op0=Alu.mult, op1=Alu.mult,
                            accum_out=PW[:, d : d + 1],
                        )
                    u = ep.tile([K, 2], f32, tag=f"u{col}")
                    nc.vector.tensor_mul(u, PW, cs_f32(cst_s[b], "m2m"))
                    nc.vector.tensor_sub(
                        u, u, cs_f32(cst_s[b], "tmw" if col == 1 else "tmr")
                    )
                    nc.vector.tensor_reduce(
                        Q[:, col : col + 1], u, axis=Ax.X, op=Alu.add,
                        apply_absolute_value=True,
                    )
                psq = psep.tile([4, 1], f32, tag="psq")
                nc.tensor.matmul(psq, lhsT=Q, rhs=ones_f, start=True, stop=True)
                nc.scalar.copy(O[:, b : b + 1], psq)

            nc.sync.dma_start(out=out[:], in_=O)

    nc.compile()
    _module_cache["nc"] = nc
    return nc


def prep_in_maps(inputs):
    """Host-side prep: reshard dense maps (contiguous 16-channel tiles,
    hm as bf16), pack reg-L1 rows, derive mask/index constants."""
    pred_hm = np.asarray(inputs["pred_hm"], np.float32)
    pred_wh = np.asarray(inputs["pred_wh"], np.float32)
    pred_reg = np.asarray(inputs["pred_reg"], np.float32)
    hm = np.asarray(inputs["hm"], np.float32)
    wh_t = np.asarray(inputs["wh_t"], np.float32)
    reg_t = np.asarray(inputs["reg_t"], np.float32)
    reg_mask = np.asarray(inputs["reg_mask"], np.float32)
    ind = np.asarray(inputs["ind"]).astype(np.int64)
    cxcy = np.asarray(inputs["cxcy"]).astype(np.int64)
    ori_wh = np.asarray(inputs["ori_wh"]).astype(np.int64)
    cls_idx = np.asarray(inputs["cls_idx"]).astype(np.int64)

    yy = np.arange(H)
    xx = np.arange(W)
    per_img = []
    for b in range(B):
        cls = cls_idx[b]
        cx, cy = cxcy[b, :, 0], cxcy[b, :, 1]
        w = wh_t[b, :, 0].astype(np.int64)
        h = wh_t[b, :, 1].astype(np.int64)
        y0 = np.maximum(1, cy - h // 2 - 1)
        y1 = np.minimum(H - 1, cy + h // 2 + 1)
        y1 = np.maximum(y1, y0)
        x0 = np.maximum(1, cx - w // 2 - 1)
        x1 = np.minimum(W - 1, cx + w // 2 + 1)
        x1 = np.maximum(x1, x0)

        MyT = ((yy[:, None] >= y0[None, :]) & (yy[:, None] < y1[None, :]))
        Mx = ((xx[None, :] >= x0[:, None]) & (xx[None, :] < x1[:, None]))
        MxR = np.tile(Mx.astype(np.float32), (1, 4))

        aspect = w.astype(np.float32) / h.astype(np.float32)
        ori = ori_wh[b, :, 0].astype(np.float32) / ori_wh[b, :, 1].astype(np.float32)
        bad = ~((aspect > 0.5 * ori) & (aspect < 2.0 * ori))
        badw = np.where(bad, 0.5, 1.0).astype(np.float32)
        valid = reg_mask[b] * (w * h > 0).astype(np.float32)

        # unique positive pixels (duplicated centers collapse in hm)
        flat = cls * (H * W) + cy * W + cx
        _, uidx = np.unique(flat, return_index=True)
        nu = len(uidx)
        cls_u, cy_u, cx_u = cls[uidx], cy[uidx], cx[uidx]
        inY = (cy_u[None, :] >= y0[:, None]) & (cy_u[None, :] < y1[:, None])
        inX = (cx_u[None, :] >= x0[:, None]) & (cx_u[None, :] < x1[:, None])
        sameC = cls[:, None] == cls_u[None, :]
        Mkj = (sameC & inY & inX).astype(np.float32)
        npos = Mkj.sum(1)
        MT = np.zeros((K, K), np.float32)
        MT[:nu, :] = Mkj.T
        # pos-pixel row in phm's [b, g, y, c16, x] flat view
        rpos_v = np.zeros((K, 1), np.int32)
        bl = b % NB
        rpos_v[:nu, 0] = (
            ((bl * NDG + cls_u // DG) * H + cy_u) * DG + (cls_u % DG)
        )
        cxsel_v = np.zeros((K, W), np.float32)
        cx_pad = np.zeros(K, np.int64)
        cx_pad[:nu] = cx_u
        cxsel_v[np.arange(K), cx_pad] = 1.0

        r = np.where(npos > 0, 1.0 / np.maximum(npos, 1.0), 1.0)
        s = (-(r * badw * valid)).astype(np.float32)

        rind = ind[b] // W
        cind = ind[b] % W
        rwr_v = ((bl * H + rind)).astype(np.int32).reshape(K, 1)
        csind_v = np.zeros((K, W), np.float32)
        csind_v[np.arange(K), cind] = 1.0

        # dma_gather indices: idx[k] = cls[k]*128 + k, wrapped [128, 8]
        idx_flat = (cls * 128 + np.arange(K)).astype(np.int16)
        gidx_v = np.zeros((128, K // 16), np.int16)
        for p in range(128):
            for s_ in range(K // 16):
                gidx_v[p, s_] = idx_flat[s_ * 16 + (p % 16)]

        m = reg_mask[b]
        M2 = np.stack([m, m], 1).astype(np.float32)
        TMW = (wh_t[b] * m[:, None]).astype(np.float32)
        TMR = (reg_t[b] * m[:, None]).astype(np.float32)
        nobj = float(m.sum())
        c1 = (1.0 / max(nobj, 1.0)) if nobj > 0 else 1.0
        invden = 1.0 / (2.0 * nobj + 1e-4)

        # pack consts [128, CST_N] f32
        cpack = np.zeros((128, CST_N), np.float32)

        def put_bf16(name, arr):
            o, n = _CST_COLS[name]
            a = np.ascontiguousarray(
                np.asarray(arr, np.float32).astype(BF16)
            )  # [128, 2n] bf16
            cpack[:, o : o + n] = a.view(np.float32)

        put_bf16("myt", MyT)
        put_bf16("mxr", MxR)
        put_bf16("mxt", np.ascontiguousarray(Mx.T))
        put_bf16("mts", MT)
        put_bf16("cxsel", cxsel_v)
        put_bf16("csind", csind_v)

        def put_f32(name, arr):
            o, n = _CST_COLS[name]
            cpack[:, o : o + n] = arr.reshape(128, n)

        put_f32("sk", s.reshape(K, 1))
        put_f32("m2m", M2)
        put_f32("tmw", TMW)
        put_f32("tmr", TMR)
        o, n = _CST_COLS["rpos"]
        cpack[:, o : o + n] = rpos_v.view(np.float32)
        o, n = _CST_COLS["rwr"]
        cpack[:, o : o + n] = rwr_v.view(np.float32)
        o, n = _CST_COLS["gidx"]
        cpack[:, o : o + n] = gidx_v.view(np.float32)

        per_img.append(dict(cpack=cpack, c1=c1, invden=invden))

    in_maps = []
    for core in range(NCORES):
        bs = [core * NB + j for j in range(NB)]
        phm_c = pred_hm[bs]  # [NB, C, H, W]
        phm_t = np.ascontiguousarray(
            phm_c.reshape(NB, NDG, DG, H, W).transpose(0, 1, 3, 2, 4)
        )
        hm_c = hm[bs]
        hm_t = np.ascontiguousarray(
            hm_c.reshape(NB, NDG, DG, H, W).transpose(0, 1, 3, 2, 4)
        ).astype(BF16)
        # pwr: [NB, H, 4, W] = (wh d0, wh d1, reg d0, reg d1) per y-row
        pwr = np.ascontiguousarray(
            np.concatenate(
                [
                    pred_wh[bs].transpose(0, 2, 1, 3),
                    pred_reg[bs].transpose(0, 2, 1, 3),
                ],
                axis=2,
            )
        )
        in_maps.append(
            {
                "phm": phm_t,
                "hm": hm_t,
                "pwr": pwr,
                "cst": np.stack([per_img[b]["cpack"] for b in bs]),
            }
        )
    aux = dict(
        c1=np.array([p["c1"] for p in per_img]),
        invden=np.array([p["invden"] for p in per_img]),
    )
    return in_maps, aux


def combine_outputs(outs, aux):
    """outs: list of 8 per-core 'out' arrays [4, NB]."""
    q = np.concatenate([o.T for o in outs], 0).astype(np.float64)  # [B, 4]
    q_hm, q_wh, q_rg = q[:, 0], q[:, 1], q[:, 2]
    wh_i = q_wh * aux["invden"]
    off_i = q_rg * aux["invden"]
    final_loss = np.mean(HM_W * q_hm + WH_W * wh_i + OFF_W * off_i)
    final_hm = np.mean(q_hm * aux["c1"])
    final_wh = np.mean(wh_i)
    final_off = np.mean(off_i)
    return (
        np.float32(final_loss),
        np.float32(final_hm),
        np.float32(final_wh),
        np.float32(final_off),
    )


def kernel(**inputs):
    from concourse.bass_utils import run_bass_kernel_spmd

    nc = build_module()
    in_maps, aux = prep_in_maps(inputs)
    res = run_bass_kernel_spmd(nc, in_maps, core_ids=list(range(NCORES)))
    outs = [r["out"] for r in res.results]
    return combine_outputs(outs, aux)
